# revision 19
# baseline (speedup 1.0000x reference)
"""Multi-head causal self-attention for TRN2, 8 NeuronCores.

Sharding: core i handles (batch b = i//2, head-group g = i%2); each head-group
is 8 of the 16 heads.  Per core everything is computed in "transposed" space so
no on-device transposes are needed.

v4 changes vs the 370us v2 (measured ~300us, HW clock varies run-to-run):
  - startup: weight/x DMAs moved off the gpsimd queue (which was blocked 22us
    by memsets before issuing any weight DMA) onto the sync queue, batched
    into ~11 large DMAs (wqk halves/wv/wp/x-per-j as single [128, wide]
    tiles).  The gpsimd queue now only runs the qTp-pad + vs-ones memsets,
    fully overlapped with the DMA stream.  First matmul ~15us instead of 41us.
  - phase-1 PSUM->SBUF bias adds moved from DVE to ACT (Identity + per-
    partition bias AP); ACT is idle in phase 1 and Exp/Identity share one
    activation table so there is no table reload.
  - V bias applied by the (single, strided) DVE scalar_tensor_tensor staging
    op per tk-chunk instead of an extra ones-row matmul per V group
    (8 CASTs -> 1 STT, and -8192 PE cycles).
  - attention: per-chunk exp is ONE ACT instr (2-region AP for diag strips),
    causal mask is ONE DVE tensor_tensor per diag chunk (2-region AP).
  - PV pipeline (depth 4) carried ACROSS head-pair units so the PE never
    drains at unit boundaries; po tiles are [128,2,TQ] pair tiles (2 banks).
  - normalization: one sums copy + one reciprocal_approx_fast per pair
    (both heads batched in [1, 2*TQ]), Pool partition_broadcast, 2 DVE
    mults; norm callbacks pop only on non-diag chunks (a broadcast-blocked
    cb4 must never head-block a mask multiply in the strict-FIFO DVE queue)
    with two no-op spacer slots between cb3 and cb4.
  - ACT trails PE by a fixed ~250ns/chunk in attention, so the PE is fed
    filler between units: the j>=2 V-projection groups are deferred into
    the first 8 attention units (borrowing ss-pool PSUM), and proj bursts
    run as 3-step slices after each unit from (j,2) on.
  - PSUM: ss pool 2x[128,2,TQ] (4 banks) + po pool 2x[128,2,TQ] (4 banks).

Rejected experiments (measured): fp8-e4m3 QKV projection via DoubleRow
(rel err 4e-2 > 2e-2 gate; V-only 3e-2, fp8 q/k 2.1e-2 -- all fail, the
kernel stays bf16 everywhere); causal masks on Pool (bcast head-blocks
masks in Pool's FIFO, 439us); quarter-split first DMAs (mid-group stall).
"""

import numpy as np
import ml_dtypes
from contextlib import ExitStack

import concourse.bass as bass
import concourse.mybir as mybir
import concourse.tile as tile
from concourse import bacc
from concourse.bass_utils import run_bass_kernel_spmd

B, T, D, H = 4, 2048, 1024, 16
DK = 64            # head dim
HL = 8             # heads per core
DL = HL * DK       # 512 local head dims per core
N_CORES = 8

F32 = mybir.dt.float32
BF16 = mybir.dt.bfloat16
EXP = mybir.ActivationFunctionType.Exp
IDENT = mybir.ActivationFunctionType.Identity
ADD = mybir.AluOpType.add

TQ = 512           # tq block size
TKC = 128          # tk chunk size
NQB = T // TQ      # 4
NKC = T // TKC     # 16
NDCH = D // 128    # 8 contraction chunks over D
VSW = HL * 65 + 65  # staged-V width: 8*[V_h|1] + tail pad for M=128 lhsT

_CACHE = {}


def _build(causal: bool):
    nc = bacc.Bacc("TRN2", target_bir_lowering=False, debug=False,
                   num_devices=N_CORES)
    x_d = nc.dram_tensor("xp", [128, NQB * NDCH * TQ], BF16,
                         kind="ExternalInput").ap()
    wqk_d = nc.dram_tensor("wqk", [128, NDCH * 1024], BF16,
                           kind="ExternalInput").ap()
    wv_d = nc.dram_tensor("wv", [128, NDCH * DL], BF16,
                          kind="ExternalInput").ap()
    wp_d = nc.dram_tensor("wproj", [128, 4 * 1024], BF16,
                          kind="ExternalInput").ap()
    bqk_d = nc.dram_tensor("bqk", [128, 8], F32, kind="ExternalInput").ap()
    bvb_d = nc.dram_tensor("bvb", [128, DL], F32, kind="ExternalInput").ap()
    tril_d = nc.dram_tensor("tril2", [TKC, 2 * TKC], BF16,
                            kind="ExternalInput").ap()
    out_d = nc.dram_tensor("out", [T, D], F32, kind="ExternalOutput").ap()

    with tile.TileContext(nc) as tc, ExitStack() as top:
        persist = top.enter_context(tc.tile_pool(name="persist", bufs=1))

        qTp = [persist.tile([128, T], BF16, tag=f"qTp{h}", name=f"qTp{h}")
               for h in range(HL)]      # per-head, zero-padded other half
        kT = [persist.tile([128, T], BF16, tag=f"kT{i}", name=f"kT{i}")
              for i in range(4)]        # head-pair packed
        vs = [persist.tile([128, VSW], BF16, tag=f"vs{t}", name=f"vs{t}")
              for t in range(NKC)]
        yT = [persist.tile([128, T], BF16, tag=f"yT{i}", name=f"yT{i}")
              for i in range(4)]
        wp_sb = persist.tile([128, 4 * 1024], BF16, tag="wp", name="wp")
        bqk_sb = persist.tile([128, 8], F32, tag="bqk", name="bqk")
        bvb_sb = persist.tile([128, DL], F32, tag="bvb", name="bvb")
        tril2 = persist.tile([TKC, 2 * TKC], BF16, tag="tril2", name="tril2")
        tril3 = tril2[:, :].rearrange("p (r c) -> p r c", r=2)

        # x and wv must outlive phase 1: the j>=2 V-projection groups are
        # deferred into early attention units as PE filler.
        xpool = top.enter_context(tc.tile_pool(name="xpool", bufs=1))
        wpool = top.enter_context(tc.tile_pool(name="wpool", bufs=1))
        wqk_sb = wpool.tile([128, NDCH * 1024], BF16, tag="wqk",
                            name="wqk_sb")
        wv_sb = wpool.tile([128, NDCH * DL], BF16, tag="wv", name="wv_sb")
        xsb = [xpool.tile([128, NDCH * TQ], BF16, tag=f"x{j}",
                          name=f"x{j}") for j in range(NQB)]

        with ExitStack() as ph1:
            ps1 = ph1.enter_context(tc.tile_pool(name="ps1", bufs=3,
                                                 space="PSUM"))
            psv = ph1.enter_context(tc.tile_pool(name="psv", bufs=2,
                                                 space="PSUM"))

            # ---- DMA issue order (sync queue; gpsimd stays free) ----
            HW = NDCH * 1024 // 2
            nc.sync.dma_start(wqk_sb[:, 0:HW], wqk_d[:, 0:HW])
            nc.sync.dma_start(xsb[0][:], x_d[:, 0:NDCH * TQ])
            nc.sync.dma_start(wqk_sb[:, HW:2 * HW], wqk_d[:, HW:2 * HW])
            nc.sync.dma_start(bqk_sb[:], bqk_d)
            nc.sync.dma_start(wv_sb[:], wv_d)
            nc.sync.dma_start(xsb[1][:], x_d[:, NDCH * TQ:2 * NDCH * TQ])
            nc.sync.dma_start(wp_sb[:], wp_d)
            nc.sync.dma_start(bvb_sb[:], bvb_d)
            if causal:
                nc.sync.dma_start(tril2[:], tril_d)
            nc.sync.dma_start(xsb[2][:], x_d[:, 2 * NDCH * TQ:3 * NDCH * TQ])
            nc.sync.dma_start(xsb[3][:], x_d[:, 3 * NDCH * TQ:4 * NDCH * TQ])

            # ---- memsets on the (otherwise idle) gpsimd queue ----
            for h in range(HL):
                pad = slice(64, 128) if h % 2 == 0 else slice(0, 64)
                nc.gpsimd.memset(qTp[h][pad, :], 0.0)
            for t in range(NKC):
                v3 = vs[t][:, 0:HL * 65].rearrange("p (h w) -> p h w", h=HL)
                nc.gpsimd.memset(v3[:, :, 64:65], 1.0)
                # tail pad read by the h=7 PV lhsT; keep it finite
                nc.gpsimd.memset(vs[t][:, HL * 65:VSW], 0.0)

            # ---------------- phase 1: QKV projections ----------------
            # V groups for j>=2 are deferred into early attention units as
            # PE filler (they borrow ss-pool PSUM there).
            def v_group(j, tt, ps, psflat):
                c = tt % 4
                for d in range(NDCH):
                    nc.tensor.matmul(
                        psflat,
                        xsb[j][:, d * TQ + c * 128:d * TQ + (c + 1) * 128],
                        wv_sb[:, d * DL:(d + 1) * DL],
                        start=(d == 0), stop=(d == NDCH - 1))
                vdst = vs[tt][:, 0:HL * 65].rearrange(
                    "p (h w) -> p h w", h=HL)[:, :, 0:64]
                psrc = psflat.rearrange("p (h w) -> p h w", h=HL)
                bsrc = bvb_sb[:].rearrange("p (h w) -> p h w", h=HL)
                nc.vector.scalar_tensor_tensor(
                    out=vdst, in0=psrc, scalar=0.0, in1=bsrc,
                    op0=ADD, op1=ADD)

            for j in range(NQB):
                jsl = slice(j * TQ, (j + 1) * TQ)
                for m in range(8):
                    ps = ps1.tile([128, TQ], F32, tag="psqk",
                                  name=f"psqk{j}_{m}")
                    for d in range(NDCH):
                        nc.tensor.matmul(
                            ps[:],
                            wqk_sb[:, d * 1024 + m * 128:
                                   d * 1024 + (m + 1) * 128],
                            xsb[j][:, d * TQ:(d + 1) * TQ],
                            start=(d == 0), stop=(d == NDCH - 1))
                    if m < 4:
                        nc.scalar.activation(
                            qTp[2 * m][0:64, jsl], ps[0:64, :], IDENT,
                            bias=bqk_sb[0:64, m:m + 1])
                        nc.scalar.activation(
                            qTp[2 * m + 1][64:128, jsl], ps[64:128, :], IDENT,
                            bias=bqk_sb[64:128, m:m + 1])
                    else:
                        nc.scalar.activation(
                            kT[m - 4][:, jsl], ps[:], IDENT,
                            bias=bqk_sb[:, m:m + 1])

                if j < 2:
                    for tt in range(4 * j, 4 * j + 4):
                        ps = psv.tile([128, DL], F32, tag="psv",
                                      name=f"psv{tt}")
                        v_group(j, tt, ps, ps[:])

        # -------- phase 2: attention + projection --------
        with ExitStack() as ph2:
            ps_s = ph2.enter_context(tc.tile_pool(name="ps_s", bufs=2,
                                                  space="PSUM"))
            ps_o = ph2.enter_context(tc.tile_pool(name="ps_o", bufs=2,
                                                  space="PSUM"))
            ppool = ph2.enter_context(tc.tile_pool(name="ppool", bufs=10))
            npool = ph2.enter_context(tc.tile_pool(name="npool", bufs=2))
            opool = ph2.enter_context(tc.tile_pool(name="opool", bufs=3))

            pending = []      # (block_j, callback)

            def pop_pending():
                if pending:
                    jb, cb = pending.pop(0)
                    if cb is not None:
                        cb()

            def flush_block(jb):
                while any(p[0] == jb for p in pending):
                    pop_pending()

            def norm_cbs(j, i, po):
                """Deferred normalization of head pair (2i, 2i+1), block j:
                divide po rows 0:64 by the softmax sums in row 64."""
                jsl = slice(j * TQ, (j + 1) * TQ)
                sumAB = npool.tile([1, 2 * TQ], F32, tag="sumAB",
                                   name=f"sa{j}_{i}")
                recAB = npool.tile([1, 2 * TQ], F32, tag="recAB",
                                   name=f"rc{j}_{i}")
                pbAB = npool.tile([64, 2 * TQ], F32, tag="pbAB",
                                  name=f"pb{j}_{i}")

                def cb1():
                    nc.vector.tensor_copy(sumAB[:], po[64:65, :, :])

                def cb2():
                    nc.vector.reciprocal_approx_fast(out=recAB[:],
                                                     in_=sumAB[:])

                def cb3():
                    nc.gpsimd.partition_broadcast(pbAB[:], recAB[:])

                def cb4():
                    nc.vector.tensor_mul(yT[i][0:64, jsl], po[0:64, 0, :],
                                         pbAB[:, 0:TQ])
                    nc.vector.tensor_mul(yT[i][64:128, jsl], po[0:64, 1, :],
                                         pbAB[:, TQ:2 * TQ])

                # the two None slots space cb4 two pop-slots after the Pool
                # broadcast so it never head-blocks the DVE queue while the
                # broadcast is still running
                return [(j, cb1), (j, cb2), (j, cb3), (j, None), (j, None),
                        (j, cb4)]

            burst_q = []      # pending proj steps: (jb, t, nb)

            def make_burst(jb):
                for t in range(4 * jb, 4 * jb + 4):
                    for nb in range(2):
                        burst_q.append((jb, t, nb))

            def run_burst_step():
                jb, t, nb = burst_q.pop(0)
                flush_block(jb)
                nsl = slice(nb * 512, (nb + 1) * 512)
                ps = ps_s.tile([128, 2, TQ], F32, tag="ss",
                               name=f"ssp{t}_{nb}")
                for k in range(4):
                    nc.tensor.matmul(
                        ps[:, 0, :], yT[k][:, t * 128:(t + 1) * 128],
                        wp_sb[:, k * 1024 + nb * 512:
                              k * 1024 + (nb + 1) * 512],
                        start=(k == 0), stop=(k == 3))
                ot = opool.tile([128, TQ], F32, tag="ot", name=f"ot{t}_{nb}")
                nc.vector.tensor_copy(ot[:], ps[:, 0, :])
                nc.sync.dma_start(out_d[t * 128:(t + 1) * 128, nsl], ot[:])
                pop_pending()

            # cross-unit PV pipeline
            pendq = []   # dicts: po, c, pt, q0, start, stop, j, i

            def emit_pv(e, extra_stop=False):
                stop = e["stop"] or extra_stop
                hA, hB = 2 * e["i"], 2 * e["i"] + 1
                q0, c, pt, po = e["q0"], e["c"], e["pt"], e["po"]
                nc.tensor.matmul(
                    po[:, 0, q0:TQ], vs[c][:, hA * 65:hA * 65 + 128],
                    pt[:, 0, q0:TQ], start=e["start"], stop=stop,
                    skip_group_check=True)
                nc.tensor.matmul(
                    po[:, 1, q0:TQ], vs[c][:, hB * 65:hB * 65 + 128],
                    pt[:, 1, q0:TQ], start=e["start"], stop=stop,
                    skip_group_check=True)
                if e["stop"]:
                    pending.extend(norm_cbs(e["j"], e["i"], po))

            units = [(j, i) for j in range(NQB) for i in range(4)]
            for j, i in units:
                hA, hB = 2 * i, 2 * i + 1
                po = ps_o.tile([128, 2, TQ], F32, tag="po",
                               name=f"po{j}_{i}")
                cs = list(range(4 * (j + 1))) if causal else list(range(NKC))
                for ci, c in enumerate(cs):
                    diag = causal and c >= 4 * j
                    q0 = (c - 4 * j) * TKC if diag else 0
                    csl = slice(c * TKC, (c + 1) * TKC)
                    ss = ps_s.tile([128, 2, TQ], F32, tag="ss",
                                   name=f"ss{j}_{i}_{c}")
                    nc.tensor.matmul(
                        ss[:, 0, q0:TQ], kT[i][:, csl],
                        qTp[hA][:, j * TQ + q0:(j + 1) * TQ],
                        start=True, stop=True)
                    nc.tensor.matmul(
                        ss[:, 1, q0:TQ], kT[i][:, csl],
                        qTp[hB][:, j * TQ + q0:(j + 1) * TQ],
                        start=True, stop=True)
                    pt = ppool.tile([128, 2, TQ], BF16, tag="pt",
                                    name=f"pt{j}_{i}_{c}")
                    nc.scalar.activation(pt[:, :, q0:TQ], ss[:, :, q0:TQ],
                                         EXP, scale=0.125)
                    if diag:
                        nc.vector.tensor_mul(pt[:, :, q0:q0 + TKC],
                                             pt[:, :, q0:q0 + TKC],
                                             tril3)
                    pendq.append(dict(po=po, c=c, pt=pt, q0=q0,
                                      start=(ci == 0), stop=(ci == len(cs) - 1),
                                      j=j, i=i))
                    if len(pendq) > 4:
                        emit_pv(pendq.pop(0))
                    # norm callbacks enter the strict-FIFO DVE queue; only
                    # pop them on chunks with no pending mask multiply so a
                    # broadcast-blocked cb never head-blocks a mask the PE
                    # is waiting on.
                    if not diag:
                        pop_pending()

                # after-unit PE filler: deferred V groups (blocks 0-1) and
                # proj-burst slices for block j-1.
                uidx = 4 * j + i
                if uidx < 8:
                    ps = ps_s.tile([128, 2, TQ], F32, tag="ss",
                                   name=f"ssv{uidx}")
                    v_group(2 + uidx // 4, 8 + uidx, ps, ps[:, 0, :])
                if i == 2 and j >= 1:
                    make_burst(j - 1)
                for _ in range(3):
                    if burst_q:
                        run_burst_step()

            while pendq:
                emit_pv(pendq.pop(0))
            while pending:
                pop_pending()
            make_burst(NQB - 1)
            while burst_q:
                run_burst_step()

    nc.compile()
    return nc


def _get_nc(causal: bool):
    if causal not in _CACHE:
        _CACHE[causal] = _build(causal)
    return _CACHE[causal]


def _host_tril2() -> np.ndarray:
    i = np.arange(TKC)[:, None]
    jj = np.arange(TKC)[None, :]
    blk = (jj >= i).astype(np.float32)
    return np.ascontiguousarray(
        np.concatenate([blk, blk], axis=1).astype(ml_dtypes.bfloat16))


def _make_in_maps(x, W_qkv, b_qkv, W_proj):
    tril_np = _host_tril2()
    bf = ml_dtypes.bfloat16
    in_maps = []
    for core in range(N_CORES):
        b, g = core // 2, core % 2
        qc = slice(g * DL, (g + 1) * DL)
        kc = slice(D + g * DL, D + (g + 1) * DL)
        vc = slice(2 * D + g * DL, 2 * D + (g + 1) * DL)
        # x packed: [p, j*4096 + d*512 + n] = x[b, j*512+n, d*128+p]
        xp = np.ascontiguousarray(
            x[b].reshape(NQB, TQ, NDCH, 128).transpose(3, 0, 2, 1)
            .reshape(128, NQB * NDCH * TQ).astype(bf))
        wqk_loc = np.concatenate([W_qkv[:, qc], W_qkv[:, kc]], axis=1)
        wqk = np.ascontiguousarray(
            wqk_loc.reshape(NDCH, 128, 2 * DL).transpose(1, 0, 2)
            .reshape(128, NDCH * 1024).astype(bf))
        wv = np.ascontiguousarray(
            W_qkv[:, vc].reshape(NDCH, 128, DL).transpose(1, 0, 2)
            .reshape(128, NDCH * DL).astype(bf))
        wp = np.ascontiguousarray(
            W_proj[g * DL:(g + 1) * DL, :].reshape(4, 128, D)
            .transpose(1, 0, 2).reshape(128, 4 * D).astype(bf))
        bqk = np.ascontiguousarray(
            np.concatenate([b_qkv[qc], b_qkv[kc]]).reshape(8, 128).T
            .astype(np.float32))
        bvb = np.ascontiguousarray(
            np.broadcast_to(b_qkv[vc][None, :], (128, DL))
            .astype(np.float32))
        in_maps.append({
            "xp": xp, "wqk": wqk, "wv": wv, "wproj": wp,
            "bqk": bqk, "bvb": bvb, "tril2": tril_np,
        })
    return in_maps


def kernel(x, mask, W_qkv, b_qkv, W_proj, b_proj):
    x = np.asarray(x, dtype=np.float32)
    mask2d = np.asarray(mask, dtype=np.int32).reshape(T, T)
    W_qkv = np.asarray(W_qkv, dtype=np.float32)
    b_qkv = np.asarray(b_qkv, dtype=np.float32)
    W_proj = np.asarray(W_proj, dtype=np.float32)
    b_proj = np.asarray(b_proj, dtype=np.float32)

    if np.array_equal(mask2d, np.tril(np.ones((T, T), dtype=np.int32))):
        causal = True
    elif np.all(mask2d == 1):
        causal = False
    else:
        raise NotImplementedError("only causal (tril) or all-ones masks")

    nc = _get_nc(causal)
    in_maps = _make_in_maps(x, W_qkv, b_qkv, W_proj)
    res = run_bass_kernel_spmd(nc, in_maps, core_ids=list(range(N_CORES)))
    out = np.empty((B, T, D), dtype=np.float32)
    for b in range(B):
        out[b] = (res.results[2 * b]["out"] + res.results[2 * b + 1]["out"]
                  + b_proj[None, :])
    return out
